# revision 8
# baseline (speedup 1.0000x reference)
"""ArcFace layer distributed Bass kernel for 8 TRN2 NeuronCores (v4).

Math (reference):
    emb_n = embedding / ||embedding||_row          [B, D]
    w_n   = kernel / ||kernel||_col                [D, C]
    cos   = emb_n @ w_n                            [B, C]
    out   = S*cos  everywhere except out[b, labels[b]] which gets the
            arcface margin value computed from cos[b, labels[b]].

Strategy (classification-parallel, per sharding hint):
  - shard kernel columns (classes) 8 ways (pad C=10572 -> 8*1328)
  - replicate embeddings; matmul operands fp16 (f32 accumulate)
  - PE warm-up dummies hold the clock at full speed before real work
  - 8 "head" m-tiles run on RAW w as soon as the first embedding columns
    land; their PSUM is released by plain ACT fp16 copies and both norm
    scales are applied later on DVE
  - remaining 8 m-tiles use rhs pre-normalized on DVE (wn = w * ws_bc,
    broadcast by gpsimd), so their epilogue is one ACT copy with a
    per-partition rs_e scale
  - embedding row-norms from DVE square+accum over a row-major embedding
    copy; label fixup from host-gathered w[:, label] columns via small
    matmuls (margin math on-device, host places the 2048 values)
  - output written fp16; DMAs split across both HWDGE queues

B=2048, D=512, C=10572, S=64, M=0.5.
"""

import math
import os

import numpy as np

os.environ.setdefault("MYCRO_LOCAL_CACHE", "1")

import concourse.bass as bass
import concourse.bacc as bacc
import concourse.mybir as mybir
import concourse.tile as tile
from concourse.bass_utils import run_bass_kernel_spmd

# ---------------- problem constants (hardcoded; kernel.py is standalone) ----
S = 64.0
MARGIN = 0.5
B = 2048          # batch
D = 512           # feature dim
C = 10572         # classes
NCORES = 8
SHARD = 1328      # class columns per core (8*1328 = 10624 >= 10572)
W = SHARD
KT = D // 128     # 4 k-subtiles
MT = B // 128     # 16 m-tiles
BSL = B // NCORES  # 256: batch slice per core for the label fixup path

COS_M = math.cos(MARGIN)
SIN_M = math.sin(MARGIN)
MM = SIN_M * MARGIN
THRESHOLD = math.cos(math.pi - MARGIN)

F32 = mybir.dt.float32
F16 = mybir.dt.float16

NCHUNKS = [(0, 512), (512, 512), (1024, W - 1024)]
HEAD = 8
NWARM = 8


def build_nc() -> bass.Bass:
    nc = bacc.Bacc()
    w_h = nc.declare_dram_parameter("w", [D, W], F16, isOutput=False)
    embT_h = nc.declare_dram_parameter("embT", [D, B], F16, isOutput=False)
    emb_h = nc.declare_dram_parameter("emb", [B, D], F16, isOutput=False)
    ewlab_h = nc.declare_dram_parameter("ewlab", [128, 8 * BSL], F16,
                                        isOutput=False)
    out_h = nc.declare_dram_parameter("out", [B, W], F16, isOutput=True)
    fixv_h = nc.declare_dram_parameter("fixv", [BSL], F32, isOutput=True)

    with tile.TileContext(nc) as tc:
        with (
            tc.tile_pool(name="persist", bufs=1) as persist,
            tc.tile_pool(name="scratch", bufs=2) as scratch,
            tc.tile_pool(name="outp", bufs=3) as outp,
            tc.tile_pool(name="micro", bufs=2) as micro,
            tc.tile_pool(name="psum", bufs=2, space="PSUM") as psum,
        ):
            wsb_all = persist.tile([128, KT, W], F16, tag="wsb")
            et_all = persist.tile([128, KT, B], F16, tag="et")
            er = persist.tile([128, MT, D], F16, tag="er")
            ewlab_t = persist.tile([128, 8 * BSL], F16, tag="ewlab")
            wsb = [wsb_all[:, kt] for kt in range(KT)]
            et = [et_all[:, kt] for kt in range(KT)]

            def et_src(c0, c1):
                return embT_h[:, c0:c1].rearrange("(kt q) c -> q kt c", q=128)

            def er_src(h):
                return emb_h[h * 1024:(h + 1) * 1024, :].rearrange(
                    "(m q) d -> q m d", q=128)

            # SP queue: embedding columns (heads first), then emb rows A
            nc.sync.dma_start(et_all[:, :, 0:512], et_src(0, 512))
            nc.sync.dma_start(et_all[:, :, 512:1024], et_src(512, 1024))
            nc.sync.dma_start(et_all[:, :, 1024:2048], et_src(1024, 2048))
            nc.sync.dma_start(er[:, 0:8], er_src(0))
            # ACT queue: w, emb rows B, fixup columns
            nc.scalar.dma_start(
                wsb_all[:], w_h[:, :].rearrange("(kt q) c -> q kt c", q=128))
            nc.scalar.dma_start(er[:, 8:16], er_src(1))
            nc.scalar.dma_start(ewlab_t[:], ewlab_h[:, :])

            ones_col = persist.tile([128, 1], F16, tag="ones")
            nc.vector.memset(ones_col[:], 1.0)
            warm_rhs = persist.tile([128, 512], F16, tag="warm_rhs")
            nc.vector.memset(warm_rhs[:], 1.0)

            # preload the ACT sqrt table while the queues stream inputs
            tbl_t = persist.tile([1, 1], F32, tag="tbl")
            nc.vector.memset(tbl_t[:], 1.0)
            tbl_o = persist.tile([1, 1], F32, tag="tbl_o")
            nc.scalar.sqrt(tbl_o[:], tbl_t[:])

            # ------------ PE warm-up: hold the clock up -------------------
            warm_ps = psum.tile([1, 512], F32, tag="nps", name="warm_ps")
            order_pin = None
            for i in range(NWARM):
                order_pin = nc.tensor.matmul(
                    out=warm_ps[:, :], lhsT=ones_col[:, :], rhs=warm_rhs[:],
                    start=True, stop=True, skip_group_check=True,
                )

            # ------------ DVE: w squares (fp16) ---------------------------
            swp4 = scratch.tile([128, KT, W], F16, tag="swp4")
            nc.vector.tensor_tensor(out=swp4[:], in0=wsb_all[:],
                                    in1=wsb_all[:], op=mybir.AluOpType.mult)
            swa = scratch.tile([128, W], F16, tag="swa")
            nc.vector.tensor_tensor(out=swa[:], in0=swp4[:, 0],
                                    in1=swp4[:, 1], op=mybir.AluOpType.add)
            swb = scratch.tile([128, W], F16, tag="swb")
            nc.vector.tensor_tensor(out=swb[:], in0=swp4[:, 2],
                                    in1=swp4[:, 3], op=mybir.AluOpType.add)
            sw = scratch.tile([128, W], F16, tag="sw")
            nc.vector.tensor_tensor(out=sw[:], in0=swa[:], in1=swb[:],
                                    op=mybir.AluOpType.add)

            # ------------ PE: head m-tiles on raw w -----------------------
            def emit_mms(m, rhs_tiles, after):
                psC = psum.tile([128, 1536], F32, tag="psC", name="psC_%d" % m)
                first = True
                last = None
                for kt in range(KT):
                    lhsT = et[kt][:, m * 128:(m + 1) * 128]
                    for (c0, cn) in NCHUNKS:
                        last = nc.tensor.matmul(
                            out=psC[:, c0:c0 + cn], lhsT=lhsT,
                            rhs=rhs_tiles[kt][:, c0:c0 + cn],
                            start=(kt == 0), stop=(kt == KT - 1),
                        )
                        if first and after is not None:
                            tile.add_dep_helper(last.ins, after.ins,
                                                sync=False,
                                                reason="stream order")
                        first = False
                return psC, last

            head_raw = [
                persist.tile([128, W], F16, tag="hraw%d" % m,
                             name="hraw%d" % m)
                for m in range(HEAD)
            ]
            wssq_mms = []
            nps_w = []
            head_pss = []
            for m in range(HEAD):
                psC, order_pin = emit_mms(m, wsb, order_pin)
                head_pss.append(psC)
                nc.scalar.copy(out=head_raw[m][:], in_=psC[:, :W])
                if m == 4:
                    # w-ssq reductions (sw is ready by now)
                    for j, (c0, cn) in enumerate(NCHUNKS):
                        nps = psum.tile([1, 512], F32, tag="nps",
                                        name="npsw%d" % j)
                        mm = nc.tensor.matmul(
                            out=nps[:, :cn], lhsT=ones_col[:, :],
                            rhs=sw[:, c0:c0 + cn], start=True, stop=True,
                        )
                        tile.add_dep_helper(mm.ins, order_pin.ins,
                                            sync=False, reason="order")
                        order_pin = mm
                        wssq_mms.append(mm)
                        nps_w.append((nps, c0, cn))

            # 1/||w||: DVE reciprocal straight from PSUM, ACT sqrt, bcast
            rw_row = persist.tile([1, W], F32, tag="rw_row")
            for (nps, c0, cn) in nps_w:
                nc.vector.reciprocal_approx_fast(
                    out=rw_row[:, c0:c0 + cn], in_=nps[:, :cn])
            rws_row = persist.tile([1, W], F16, tag="rws_row")
            nc.scalar.sqrt(rws_row[:], rw_row[:])
            ws_bc = persist.tile([128, W], F16, tag="ws_bc")
            nc.gpsimd.partition_broadcast(ws_bc[:], rws_row[:])

            # normalized rhs tiles, pipelined by kt
            wn = [
                persist.tile([128, W], F16, tag="wn%d" % kt, name="wn%d" % kt)
                for kt in range(KT)
            ]
            for kt in range(KT):
                nc.vector.tensor_tensor(out=wn[kt][:], in0=wsb[kt][:],
                                        in1=ws_bc[:], op=mybir.AluOpType.mult)

            # ------------ DVE: e row-norm square+accum --------------------
            sq_dump = persist.tile([128, D], F16, tag="sq_dump")
            essq = persist.tile([128, MT], F32, tag="essq")
            rs_tmp = persist.tile([128, MT], F32, tag="rs_tmp")
            rs_em = persist.tile([128, MT], F32, tag="rs_em")

            def emit_rs_accums(m0, m1):
                for m in range(m0, m1):
                    nc.vector.scalar_tensor_tensor(
                        out=sq_dump[:], in0=er[:, m], scalar=1.0,
                        in1=er[:, m], op0=mybir.AluOpType.mult,
                        op1=mybir.AluOpType.mult,
                        accum_out=essq[:, m:m + 1],
                    )
                nc.vector.reciprocal_approx_fast(
                    out=rs_tmp[:, m0:m1], in_=essq[:, m0:m1])

            def emit_rs_sqrt(m0, m1):
                # rs = S/sqrt(ssq) = sqrt(S^2 / ssq)
                nc.scalar.activation(
                    rs_em[:, m0:m1], rs_tmp[:, m0:m1],
                    mybir.ActivationFunctionType.Sqrt, scale=S * S,
                )

            emit_rs_accums(8, 16)     # erB lands first (ACT queue)
            emit_rs_sqrt(8, 16)

            # ------------ fixup products (DVE) + dot matmuls (PE) ---------
            elab = ewlab_t[:, 0:4 * BSL]
            wlab = ewlab_t[:, 4 * BSL:8 * BSL]
            prod = scratch.tile([128, 4 * BSL], F16, tag="prod")
            nc.vector.tensor_tensor(out=prod[:], in0=elab, in1=wlab,
                                    op=mybir.AluOpType.mult)
            sqew = scratch.tile([128, 8 * BSL], F16, tag="sqew")
            nc.vector.tensor_tensor(out=sqew[:], in0=ewlab_t[:],
                                    in1=ewlab_t[:], op=mybir.AluOpType.mult)

            # rs accums for the head half must be EMITTED before their
            # sqrt consumer below (tile deps follow emission order); they
            # wait on the late erA DMA at runtime.
            emit_rs_accums(0, 8)

            fix_ps = {}

            def emit_fix_mms(after):
                last = after
                for name, src in (
                    ("dot", prod[:, 0:4 * BSL]),
                    ("esl", sqew[:, 0:4 * BSL]),
                    ("wsl", sqew[:, 4 * BSL:8 * BSL]),
                ):
                    ps = psum.tile([1, 512], F32, tag="nps",
                                   name="ps_%s" % name)
                    mm = nc.tensor.matmul(
                        out=ps[:, :], lhsT=ones_col[:, :],
                        rhs=src[:, 0:512], start=True, stop=False)
                    tile.add_dep_helper(mm.ins, last.ins, sync=False,
                                        reason="order")
                    last = nc.tensor.matmul(
                        out=ps[:, :], lhsT=ones_col[:, :],
                        rhs=src[:, 512:1024], start=False, stop=True)
                    fix_ps[name] = ps
                return last

            # ------------ PE mains + ACT epilogue -------------------------
            ot_pairs = {}
            pair_dma = {}

            def emit_epilogue(m, psC):
                pr, mloc = divmod(m, 2)
                if mloc == 0:
                    ot_pairs[pr] = outp.tile([128, 2, W], F16, tag="ot",
                                             name="ot%d" % pr)
                nc.scalar.mul(ot_pairs[pr][:, mloc], psC[:, :W],
                              rs_em[:, m:m + 1])
                dst = out_h[pr * 256:(pr + 1) * 256, :].rearrange(
                    "(two q) c -> q two c", q=128)
                pair_dma[pr] = (dst, ot_pairs[pr])

            for m in range(HEAD, MT):
                pss, order_pin = emit_mms(m, wn, order_pin)
                emit_epilogue(m, pss)
                if m == 9:
                    order_pin = emit_fix_mms(order_pin)
                    emit_rs_sqrt(0, 8)   # ACT: after epi m8/m9
                # main-pair DMAs on ACT right after the odd epilogue,
                # last pair as two singles to shorten the tail
                pr = (m - 1) // 2
                if m % 2 == 1 and m < 14:
                    dst, src = pair_dma[pr]
                    nc.scalar.dma_start(dst, src[:])
            # last pair: single-tile DMAs
            dst, src = pair_dma[7]
            nc.scalar.dma_start(
                out_h[1792:1920, :], src[:, 0])
            nc.scalar.dma_start(
                out_h[1920:2048, :], src[:, 1])

            # ------------ finish heads on DVE, head-pair DMAs on SP -------
            head_ots = {}
            for m in range(HEAD):
                pr, mloc = divmod(m, 2)
                if mloc == 0:
                    head_ots[pr] = outp.tile([128, 2, W], F16, tag="hot",
                                             name="hot%d" % pr)
                nc.vector.scalar_tensor_tensor(
                    out=head_ots[pr][:, mloc], in0=head_raw[m][:],
                    scalar=rs_em[:, m:m + 1], in1=ws_bc[:],
                    op0=mybir.AluOpType.mult, op1=mybir.AluOpType.mult,
                )
                if mloc == 1:
                    dst = out_h[pr * 256:(pr + 1) * 256, :].rearrange(
                        "(two q) c -> q two c", q=128)
                    nc.sync.dma_start(dst, head_ots[pr][:])

            # ------------ fixup margin math on [1, BSL] -------------------
            def half_add(name, ps, dt=F32):
                h0 = micro.tile([1, BSL], dt, tag="fx_h_" + name,
                                name=name + "_h0")
                nc.vector.tensor_copy(out=h0[:], in_=ps[:, 0:BSL])
                t = micro.tile([1, BSL], dt, tag="fx_" + name, name=name)
                nc.vector.tensor_tensor(out=t[:], in0=h0[:],
                                        in1=ps[:, BSL:2 * BSL],
                                        op=mybir.AluOpType.add)
                return t

            dot = half_add("dot", fix_ps["dot"])
            esl = half_add("esl", fix_ps["esl"])
            wsl = half_add("wsl", fix_ps["wsl"])

            sp_t = micro.tile([1, BSL], F32, tag="fx_sp")
            nc.vector.tensor_tensor(out=sp_t[:], in0=esl[:], in1=wsl[:],
                                    op=mybir.AluOpType.mult)
            rp = micro.tile([1, BSL], F32, tag="fx_rp")
            nc.vector.reciprocal_approx_fast(out=rp[:], in_=sp_t[:])
            rnorm = micro.tile([1, BSL], F32, tag="fx_rn")
            nc.scalar.sqrt(rnorm[:], rp[:])
            g = micro.tile([1, BSL], F32, tag="fx_g")
            nc.vector.scalar_tensor_tensor(
                out=g[:], in0=dot[:], scalar=S, in1=rnorm[:],
                op0=mybir.AluOpType.mult, op1=mybir.AluOpType.mult,
            )
            om = micro.tile([1, BSL], F32, tag="fx_om")
            nc.vector.scalar_tensor_tensor(
                out=om[:], in0=g[:], scalar=-1.0 / (S * S), in1=g[:],
                op0=mybir.AluOpType.mult, op1=mybir.AluOpType.mult,
            )
            nc.vector.tensor_scalar_add(om[:], om[:], 1.0)
            nc.vector.tensor_scalar_max(om[:], om[:], 0.0)
            sin_t = micro.tile([1, BSL], F32, tag="fx_sin")
            nc.scalar.sqrt(sin_t[:], om[:])
            cosmt = micro.tile([1, BSL], F32, tag="fx_cosmt")
            nc.vector.tensor_scalar_mul(cosmt[:], g[:], COS_M)
            nc.vector.scalar_tensor_tensor(
                out=cosmt[:], in0=sin_t[:], scalar=-S * SIN_M, in1=cosmt[:],
                op0=mybir.AluOpType.mult, op1=mybir.AluOpType.add,
            )
            keep = micro.tile([1, BSL], F32, tag="fx_keep")
            nc.vector.tensor_scalar_add(keep[:], g[:], -S * MM)
            mask = micro.tile([1, BSL], mybir.dt.uint8, tag="fx_mask")
            nc.vector.tensor_scalar(
                out=mask[:], in0=g[:], scalar1=S * THRESHOLD, scalar2=None,
                op0=mybir.AluOpType.is_gt,
            )
            val = micro.tile([1, BSL], F32, tag="fx_val")
            nc.vector.select(val[:], mask[:], cosmt[:], keep[:])
            nc.sync.dma_start(fixv_h[None, :], val[:])

    nc.finalize()
    return nc


_NC_CACHE: bass.Bass | None = None


def get_nc() -> bass.Bass:
    global _NC_CACHE
    if _NC_CACHE is None:
        _NC_CACHE = build_nc()
    return _NC_CACHE


def make_in_maps(embedding: np.ndarray, kernel: np.ndarray, labels: np.ndarray):
    embedding = np.asarray(embedding, dtype=np.float32)
    kernel = np.asarray(kernel, dtype=np.float32)
    labels = np.asarray(labels, dtype=np.int64)

    emb16 = embedding.astype(np.float16)
    embT = np.ascontiguousarray(emb16.T)
    kern_pad = np.ones((D, NCORES * SHARD), dtype=np.float32)
    kern_pad[:, :C] = kernel
    kern16 = kern_pad.astype(np.float16)

    in_maps = []
    for i in range(NCORES):
        wi = np.ascontiguousarray(kern16[:, i * SHARD:(i + 1) * SHARD])
        sl = slice(i * BSL, (i + 1) * BSL)
        elab = embT[:, sl].reshape(KT, 128, BSL).transpose(1, 0, 2)
        wlab = kern16[:, labels[sl]].reshape(KT, 128, BSL).transpose(1, 0, 2)
        ew = np.concatenate(
            [elab.reshape(128, KT * BSL), wlab.reshape(128, KT * BSL)], axis=1
        )
        in_maps.append(
            {
                "w": wi,
                "embT": embT,
                "emb": emb16,
                "ewlab": np.ascontiguousarray(ew),
            }
        )
    return in_maps


def assemble(results, labels) -> np.ndarray:
    parts = [
        np.asarray(results[i]["out"]).reshape(B, W) for i in range(NCORES)
    ]
    full = np.concatenate(parts, axis=1)[:, :C].astype(np.float32)
    fixv = np.concatenate(
        [np.asarray(results[i]["fixv"]).reshape(BSL) for i in range(NCORES)]
    ).astype(np.float32)
    labels = np.asarray(labels, dtype=np.int64)
    b = np.arange(B)
    # guard: valid margin values are bounded; fall back to the plain logit
    ok = np.isfinite(fixv) & (np.abs(fixv) < 2.0 * S)
    vals = np.where(ok, fixv, full[b, labels])
    full[b, labels] = vals
    return full


def kernel(embedding: np.ndarray, kernel: np.ndarray, labels: np.ndarray) -> np.ndarray:
    nc = get_nc()
    in_maps = make_in_maps(embedding, kernel, labels)
    last_err = None
    for _attempt in range(3):
        try:
            res = run_bass_kernel_spmd(nc, in_maps, core_ids=list(range(NCORES)))
            return assemble(res.results, labels)
        except Exception as e:  # transient NRT/device errors: retry
            last_err = e
    raise last_err


if __name__ == "__main__":
    rng = np.random.default_rng(0)
    emb = rng.standard_normal((B, D), dtype=np.float32)
    kern = (rng.standard_normal((D, C), dtype=np.float32) * 0.05).astype(np.float32)
    labs = rng.integers(0, C, size=(B,), dtype=np.int32)
    out = kernel(emb, kern, labs)
    print(out.shape, out.dtype)
